# revision 1
# baseline (speedup 1.0000x reference)
"""Trainium2 Bass kernel for nn_AttentionResidual (sparse_attention).

Computes, for V:(n=8,b=4,s=2048,d=1024), proj:(12,1024), scale:(1024,), block_idx:
    w       = proj[min(block_idx, 11)]
    rms     = sqrt(mean(V^2, axis=-1) + 1e-5)
    logits  = sum_d (w*scale)[d] * V[...,d] / rms          # == <w, K> with K = V/rms*scale
    weights = softmax(logits, axis=n)
    out     = sum_n weights[n] * V[n]                       # (b,s,d)

Sharding: data-parallel over the 8192 (b,s) positions across 8 NeuronCores
(1024 positions per core). proj/scale are folded into a single d-vector on the
host and broadcast. No collectives.

Per core, per pair of 128-position blocks (pairing batches the ACT table sets):
  - DMA 16 n-tiles [128pos, 1024d] f32 (contiguous 4KB/partition lines)
  - ACT: sum-of-squares per tile via Square activation with accum_out
  - DVE: ws-dot per tile via scalar_tensor_tensor with accum_out
  - softmax over n=8 on [128,8] stat tiles; rsqrt via exp(-0.5*ln(ms))
    plus one Newton refinement
  - weighted sum over n on the TensorEngine: diag(w_n) built by DVE
    tensor_scalar, then 8 accumulating fp32 matmuls per PSUM bank
    (fp32 PE matmuls are exact for diagonal weights); ACT copies
    PSUM->SBUF for the output DMA
"""

import numpy as np

N, B, S, D = 8, 4, 2048, 1024
NCORES = 8
BS = B * S            # 8192 flattened (b,s) positions
PER = BS // NCORES    # 1024 positions per core
PB = PER // 128       # 8 position blocks per core
EPS = 1e-5

_cache = {}


def _build():
    import concourse.tile as tile
    from concourse import bacc, mybir

    OP = mybir.AluOpType
    A = mybir.ActivationFunctionType
    X = mybir.AxisListType.X
    f32 = mybir.dt.float32

    from concourse.hw_specs import get_activation_tables

    nc = bacc.Bacc(
        "TRN2",
        target_bir_lowering=False,
        debug=False,
        enable_asserts=False,
        num_devices=NCORES,
    )
    v = nc.dram_tensor("v", [N, PER, D], f32, kind="ExternalInput").ap()
    wsb = nc.dram_tensor("wsb", [128, D], f32, kind="ExternalInput").ap()
    ident = nc.dram_tensor("ident", [128, 128], f32, kind="ExternalInput").ap()
    o = nc.dram_tensor("o", [PER, D], f32, kind="ExternalOutput").ap()

    # One ACT table set covers Square/Ln/Exp/Copy; pre-place its load so the
    # bacc pass doesn't ping-pong between smaller sets (one load per set
    # switch costs ~1.3us on the Scalar engine).
    act_set_id = list(get_activation_tables(nc.m.arch).keys()).index(
        "natural_log_exp_and_others"
    )

    with tile.TileContext(nc) as tc:
        with (
            tc.tile_pool(name="vp", bufs=34) as vp,
            tc.tile_pool(name="wp", bufs=1) as wp,
            tc.tile_pool(name="scr", bufs=3) as scr,
            tc.tile_pool(name="st", bufs=6) as st,
            tc.tile_pool(name="dg", bufs=18) as dgp,
            tc.tile_pool(name="ac", bufs=4) as ac,
            tc.tile_pool(name="ps", bufs=3, space="PSUM") as ps,
        ):
            nc.scalar.add_instruction(
                mybir.InstLoadActFuncSet(
                    name=nc.get_next_instruction_name(),
                    ins=[],
                    outs=[],
                    act_func_set_id=act_set_id,
                )
            )
            wt = wp.tile([128, D], f32, tag="w")
            nc.sync.dma_start(wt[:], wsb[:])
            idt = wp.tile([128, 128], f32, tag="id")
            nc.sync.dma_start(idt[:], ident[:])
            epsb = wp.tile([128, 1], f32, tag="eps")
            nc.vector.memset(epsb[:], EPS)

            for pp in range(PB):  # per position block
                pbs = (pp,)
                vts = {}
                for pb in pbs:
                    lo = pb * 128
                    for n in range(N):
                        t = vp.tile([128, D], f32, tag="v", name=f"v_{pb}_{n}")
                        nc.sync.dma_start(t[:], v[n, lo : lo + 128, :])
                        vts[(pb, n)] = t

                # reductions (ACT: sum V^2; DVE: sum ws*V)
                ss = {}
                dot = {}
                for pb in pbs:
                    ss[pb] = st.tile([128, N], f32, tag="ss", name=f"ss_{pb}")
                    dot[pb] = st.tile([128, N], f32, tag="dot", name=f"dot_{pb}")
                for pb in pbs:
                    for n in range(N):
                        sq = scr.tile([128, D], f32, tag="sq")
                        nc.scalar.activation(
                            sq[:], vts[(pb, n)][:], A.Square,
                            accum_out=ss[pb][:, n : n + 1],
                        )
                        td = scr.tile([128, D], f32, tag="td")
                        nc.vector.scalar_tensor_tensor(
                            out=td[:], in0=vts[(pb, n)][:], scalar=1.0, in1=wt[:],
                            op0=OP.mult, op1=OP.mult,
                            accum_out=dot[pb][:, n : n + 1],
                        )

                # softmax over n: inv_rms = exp(-0.5*ln(ss/D + eps)) (~1ulp-grade
                # for ms near 1); weights left unnormalized as e with a
                # per-partition 1/sum factor rs folded in downstream.
                lnt, y0 = {}, {}
                for pb in pbs:
                    lnt[pb] = st.tile([128, N], f32, tag="lnt", name=f"lnt_{pb}")
                    nc.scalar.activation(
                        lnt[pb][:], ss[pb][:], A.Ln, bias=epsb[:], scale=1.0 / D
                    )
                for pb in pbs:
                    y0[pb] = st.tile([128, N], f32, tag="y0", name=f"y0_{pb}")
                    nc.scalar.activation(y0[pb][:], lnt[pb][:], A.Exp, scale=-0.5)
                ecol, rcol = {}, {}
                for pb in pbs:
                    lg = st.tile([128, N], f32, tag="lg")
                    nc.vector.tensor_mul(lg[:], dot[pb][:], y0[pb][:])
                    nm = st.tile([128, 1], f32, tag="nm")
                    nc.vector.tensor_reduce(nm[:], lg[:], X, OP.max, negate=True)
                    e = st.tile([128, N], f32, tag="e", name=f"e_{pb}")
                    sume = st.tile([128, 1], f32, tag="sume")
                    nc.scalar.activation(
                        e[:], lg[:], A.Exp, bias=nm[:], accum_out=sume[:]
                    )
                    rs = st.tile([128, 1], f32, tag="rs", name=f"rs_{pb}")
                    nc.vector.reciprocal(rs[:], sume[:])
                    ecol[pb], rcol[pb] = e, rs

                # weighted sum over n: TensorEngine for most blocks
                # (psum[:, bank] += diag(e_n/sum) @ V_n; fp32 PE is exact for
                # diagonals), VectorE MAC chain for the rest to balance load.
                for pb in pbs:
                    e, rs = ecol[pb], rcol[pb]
                    if pb not in (0, PB - 1):  # TensorEngine path (6 of 8 blocks)
                        diags = []
                        if pb in (1, 4, 6):
                            # build diags on ACT (scale = normalized weight col)
                            wc = st.tile([128, N], f32, tag="wc", name=f"wc_{pb}")
                            nc.scalar.activation(wc[:], e[:], A.Copy, scale=rs[:])
                            for n in range(N):
                                dg = dgp.tile([128, 128], f32, tag="dg")
                                nc.scalar.activation(
                                    dg[:], idt[:], A.Copy, scale=wc[:, n : n + 1]
                                )
                                diags.append(dg)
                        else:
                            for n in range(N):
                                dg = dgp.tile([128, 128], f32, tag="dg")
                                nc.vector.tensor_scalar(
                                    dg[:], idt[:], e[:, n : n + 1], rs[:],
                                    OP.mult, OP.mult,
                                )
                                diags.append(dg)
                        acc_ps = ps.tile([128, D], f32, tag="acc")
                        for n in range(N):
                            nc.tensor.matmul(
                                acc_ps[:, 0:512], diags[n][:], vts[(pb, n)][:, 0:512],
                                start=(n == 0), stop=(n == N - 1),
                            )
                            nc.tensor.matmul(
                                acc_ps[:, 512:1024], diags[n][:],
                                vts[(pb, n)][:, 512:1024],
                                start=(n == 0), stop=(n == N - 1),
                            )
                        acc = ac.tile([128, D], f32, tag="acc_sb")
                        nc.scalar.copy(acc[:], acc_ps[:])
                    else:  # VectorE MAC chain on unnormalized e, then scale by rs
                        acc = ac.tile([128, D], f32, tag="acc_sb")
                        nc.vector.tensor_scalar(
                            acc[:], vts[(pb, 0)][:], e[:, 0:1], None, OP.mult
                        )
                        for n in range(1, N):
                            nc.vector.scalar_tensor_tensor(
                                out=acc[:], in0=vts[(pb, n)][:],
                                scalar=e[:, n : n + 1], in1=acc[:],
                                op0=OP.mult, op1=OP.add,
                            )
                        nc.vector.tensor_scalar(acc[:], acc[:], rs[:], None, OP.mult)
                    nc.sync.dma_start(o[pb * 128 : (pb + 1) * 128, :], acc[:])

    nc.compile()
    return nc


def get_program():
    if "nc" not in _cache:
        _cache["nc"] = _build()
    return _cache["nc"]


def make_in_maps(V, proj, scale, block_idx):
    V = np.asarray(V, dtype=np.float32)
    proj = np.asarray(proj, dtype=np.float32)
    scale = np.asarray(scale, dtype=np.float32)
    idx = min(int(block_idx), proj.shape[0] - 1)
    ws = (proj[idx] * scale).astype(np.float32)
    wsb = np.ascontiguousarray(np.broadcast_to(ws, (128, D)))
    eye = np.eye(128, dtype=np.float32)
    Vf = V.reshape(N, BS, D)
    return [
        {
            "v": np.ascontiguousarray(Vf[:, k * PER : (k + 1) * PER, :]),
            "wsb": wsb,
            "ident": eye,
        }
        for k in range(NCORES)
    ]


def kernel(V, proj, scale, block_idx):
    from concourse.bass_utils import run_bass_kernel_spmd

    nc = get_program()
    in_maps = make_in_maps(V, proj, scale, block_idx)
    res = run_bass_kernel_spmd(nc, in_maps, core_ids=list(range(NCORES)))
    _cache["last_exec_time_ns"] = res.exec_time_ns
    _cache["last_results"] = res
    out = np.concatenate([res.results[k]["o"] for k in range(NCORES)], axis=0)
    return out.reshape(B, S, D)

